# revision 8
# baseline (speedup 1.0000x reference)
"""Trainium2 Bass kernel for the LoRA-update contraction.

Computes out[b,n] = sum_l <B_l @ A_l, gradient[l,b,n]>_F for
  lora_A    [48, 8, 1024]       (L, R, IN)
  lora_B    [48, 1024, 8]       (L, OUT, R)
  gradient  [48, 4, 2, 1024, 1024]  (L, B, N, OUT, IN)

Strategy (memory-bound problem — gradient is 1.6 GB):
  - Shard L across the 8 NeuronCores (6 layers each, 201 MB of gradient per
    core). Per-core partial outputs [B*N] are summed on the host.
  - On each core: W_l = B_l @ A_l is computed once per layer on the
    TensorEngine (fp32, cheap: 50M MACs), then the gradient streams through
    SBUF in 4 MB blocks and a fused VectorEngine tensor_tensor_reduce does
    acc[p] += sum_i G[p,i]*W[p,i] in a single pass at full fp32 precision.
  - A final ones-vector matmul reduces the 128 partition accumulators.
"""

import numpy as np

L, R, OUT, IN = 48, 8, 1024, 1024
B, N = 4, 2
NCORES = 8
LP = L // NCORES  # layers per core
BN = B * N

_PART = 128


def build_module(lp=LP, bn=BN, out_dim=OUT, in_dim=IN, r=R):
    """Build + compile the per-core Bass module (same program on all cores)."""
    import concourse.bacc as bacc
    import concourse.mybir as mybir
    from concourse.tile import TileContext

    fp32 = mybir.dt.float32
    oc = out_dim // _PART          # number of 128-row chunks of OUT
    n_mm = min(512, in_dim)        # matmul moving-dim tile (one PSUM bank)
    ih = in_dim // n_mm
    # TTR chunk: cover `cw` o-chunks per op to amortize DVE op overhead
    cw = 2 if oc % 2 == 0 else 1
    nh = oc // cw

    nc = bacc.Bacc("TRN2", target_bir_lowering=False, debug=False)

    nchunk = lp * (oc // cw)
    g = nc.dram_tensor("g", [lp, bn, out_dim, in_dim], fp32, kind="ExternalInput").ap()
    bt = nc.dram_tensor("bt", [lp, r, out_dim], fp32, kind="ExternalInput").ap()
    a = nc.dram_tensor("a", [lp, r, in_dim], fp32, kind="ExternalInput").ap()
    # Per-(partition, bn, chunk) partial sums; the final reduction over
    # partitions/chunks (a few KB) happens on the host.
    out = nc.dram_tensor("out", [_PART, bn, nchunk], fp32, kind="ExternalOutput").ap()

    with TileContext(nc) as tc:
        with (
            tc.tile_pool(name="gpool", bufs=3) as gpool,
            tc.tile_pool(name="wpool", bufs=2) as wpool,
            tc.tile_pool(name="abpool", bufs=2) as abpool,
            tc.tile_pool(name="spool", bufs=2) as spool,
            tc.tile_pool(name="small", bufs=1) as small,
            tc.tile_pool(name="pspool", bufs=4, space="PSUM") as pspool,
        ):
            acc = small.tile([_PART, bn, nchunk], fp32)

            for l in range(lp):
                # Per-layer LoRA factors: bt[l] is B^T (r x out), a[l] is (r x in)
                bt_t = abpool.tile([r, out_dim], fp32, tag="bt")
                nc.sync.dma_start(out=bt_t[:], in_=bt[l])
                a_t = abpool.tile([r, in_dim], fp32, tag="a")
                nc.sync.dma_start(out=a_t[:], in_=a[l])

                # W_l[o, i] = sum_r B[o,r] A[r,i]; stored as [128, oc, in]
                w = wpool.tile([_PART, oc, in_dim], fp32, tag="w")
                for c in range(oc):
                    for h in range(ih):
                        ps = pspool.tile([_PART, n_mm], fp32, tag="ps")
                        nc.tensor.matmul(
                            ps[:],
                            lhsT=bt_t[:, c * _PART:(c + 1) * _PART],
                            rhs=a_t[:, h * n_mm:(h + 1) * n_mm],
                            start=True,
                            stop=True,
                        )
                        nc.scalar.copy(
                            out=w[:, c, h * n_mm:(h + 1) * n_mm], in_=ps[:]
                        )

                for j in range(bn):
                    gt = gpool.tile([_PART, oc, in_dim], fp32, tag="g")
                    g_src = g[l, j].rearrange("(c p) i -> p c i", p=_PART)
                    last = l == lp - 1 and j == bn - 1
                    if last:
                        # Split the final block's DMA per STT chunk so the
                        # tail STTs start on partial data.
                        for h in range(nh):
                            nc.sync.dma_start(
                                out=gt[:, h * cw:(h + 1) * cw, :],
                                in_=g_src[:, h * cw:(h + 1) * cw, :],
                            )
                    else:
                        nc.sync.dma_start(out=gt[:], in_=g_src)
                    for h in range(nh):
                        sc = spool.tile([_PART, cw, in_dim], fp32, tag="sc")
                        nc.vector.scalar_tensor_tensor(
                            out=sc[:],
                            in0=gt[:, h * cw:(h + 1) * cw, :],
                            scalar=1.0,
                            in1=w[:, h * cw:(h + 1) * cw, :],
                            op0=mybir.AluOpType.mult,
                            op1=mybir.AluOpType.mult,
                            accum_out=acc[:, j, l * nh + h:l * nh + h + 1],
                        )

            nc.sync.dma_start(out=out[:], in_=acc[:])

    nc.compile()
    return nc


_NC_CACHE = {}


def _get_module():
    if "nc" not in _NC_CACHE:
        _NC_CACHE["nc"] = build_module()
    return _NC_CACHE["nc"]


def make_in_maps(lora_A, lora_B, gradient):
    lora_A = np.asarray(lora_A, dtype=np.float32)
    lora_B = np.asarray(lora_B, dtype=np.float32)
    gradient = np.asarray(gradient, dtype=np.float32)
    in_maps = []
    for c in range(NCORES):
        sl = slice(LP * c, LP * (c + 1))
        in_maps.append({
            "g": np.ascontiguousarray(gradient[sl].reshape(LP, BN, OUT, IN)),
            "bt": np.ascontiguousarray(lora_B[sl].transpose(0, 2, 1)),
            "a": np.ascontiguousarray(lora_A[sl]),
        })
    return in_maps


def kernel(lora_A, lora_B, gradient, _trace=False, _trace_kwargs=None):
    from concourse.bass_utils import run_bass_kernel_spmd

    nc = _get_module()
    in_maps = make_in_maps(lora_A, lora_B, gradient)
    last_exc = None
    for attempt in range(3):
        try:
            res = run_bass_kernel_spmd(
                nc,
                in_maps,
                core_ids=list(range(NCORES)),
                trace=_trace,
                **(_trace_kwargs or {}),
            )
            break
        except Exception as e:  # transient device wedges (NRT_EXEC_UNIT_...)
            last_exc = e
            import time as _time

            _time.sleep(15 * (attempt + 1))
    else:
        raise last_exc
    total = np.zeros(BN, np.float64)
    for m in res.results:
        total += m["out"].astype(np.float64).sum(axis=(0, 2))
    out = total.astype(np.float32).reshape(B, N)
    if _trace:
        return out, res
    return out
